# revision 19
# baseline (speedup 1.0000x reference)
"""Trainium2 Bass kernel for the GCAGNN (nn_GCAGNN_14843406975360).

Strategy (8 NeuronCores, SPMD):
  * Every pga_linear is a dense matmul with a host-precomputed (I*16, O*16)
    matrix. The message MLP's first linear commutes with the edge gather
    (h1[e] = A[dst] + B[src], A = X@Wd, B = X@Ws) and its last linear
    commutes with the scatter-add (agg = segsum(h2g) @ W2, with W2@U0a
    folded into the update MLP on the host).
  * Edges sorted by dst; cores own contiguous 128-aligned node-tile ranges,
    so per-core segment sums are complete — no all-reduce. Between layers
    only the per-node B table is all-gathered (packed bf16 hi/lo split,
    ~2^-16 accurate).
  * Gather = dma_gather (transpose mode, bf16, feature-major output),
    512 indices per call (the Q7 single-packet path tops out at 64
    descriptors/engine — 1024-idx gathers crash the device).
    Scatter-add = one-hot matmul accumulated in PSUM per 512-edge group,
    then dma_scatter_add into a DRAM accumulator (row->engine mapping is
    index-position-fixed, so repeated rows serialize per engine: race-free).
"""
import itertools
import sys
import types
import numpy as np

N_NODES, N_EDGES, NCORES = 10000, 160000, 8
GRP = 512            # edge group size (matmul N)
GCH = 512            # indices per dma_gather chunk (64-desc packet limit)
D_LAYERS = [128, 256, 256, 128]
H = 256

_METRIC = [0.0, 1.0, 1.0, 1.0]
ACTION_BLADES = np.array([0, 5, 6, 7, 8, 9, 10, 15])


def _build_algebra():
    blades = [b for g in range(5) for b in itertools.combinations(range(4), g)]
    index = {b: i for i, b in enumerate(blades)}
    C = np.zeros((16, 16, 16), dtype=np.float32)
    for i, a in enumerate(blades):
        for j, b in enumerate(blades):
            coeff, res = 1.0, list(a)
            for g in b:
                coeff *= (-1.0) ** sum(1 for h in res if h > g)
                if g in res:
                    coeff *= _METRIC[g]
                    res.remove(g)
                else:
                    res.append(g)
                    res.sort()
            if coeff != 0.0:
                C[i, j, index[tuple(res)]] += coeff
    rev = np.array([(-1.0) ** (len(b) * (len(b) - 1) // 2) for b in blades], np.float32)
    return C, rev


def _big_linear(w, a8, CAYLEY, REV):
    O, I = np.asarray(w).shape
    a = np.zeros((O, I, 16), np.float32)
    a[:, :, ACTION_BLADES] = np.asarray(a8, np.float32)
    ar = a * REV
    T = np.einsum("oip,pqr->oiqr", a, CAYLEY)
    K = np.einsum("oiqr,ois,rsk->oiqk", T, ar, CAYLEY)
    M = np.einsum("oi,oiqk->iqok", np.asarray(w, np.float32), K)
    return M.reshape(I * 16, O * 16)


def _gate_mats(kw, kb):
    kw = np.asarray(kw, np.float32)
    kb = np.asarray(kb, np.float32)
    C = kw.shape[0]
    GR = np.zeros((C * 16, C * 16), np.float32)
    for c in range(C):
        GR[c * 16 : (c + 1) * 16, c * 16 : (c + 1) * 16] = np.tile(
            kw[c][:, None], (1, 16)
        )
    return GR, np.repeat(kb, 16)


def _prep_params(params):
    CAYLEY, REV = _build_algebra()
    out = []
    for layer in params:
        m, u = layer["message"], layer["update"]
        W0 = _big_linear(m["linears"][0]["w"], m["linears"][0]["a"], CAYLEY, REV)
        ci16 = W0.shape[0] // 2
        W1 = _big_linear(m["linears"][1]["w"], m["linears"][1]["a"], CAYLEY, REV)
        W2 = _big_linear(m["linears"][2]["w"], m["linears"][2]["a"], CAYLEY, REV)
        U0 = _big_linear(u["linears"][0]["w"], u["linears"][0]["a"], CAYLEY, REV)
        U1 = _big_linear(u["linears"][1]["w"], u["linears"][1]["a"], CAYLEY, REV)
        U2 = _big_linear(u["linears"][2]["w"], u["linears"][2]["a"], CAYLEY, REV)
        GRm1, bm1 = _gate_mats(m["acts"][0]["kw"], m["acts"][0]["kb"])
        GRu1, bu1 = _gate_mats(u["acts"][0]["kw"], u["acts"][0]["kb"])
        GRu2, bu2 = _gate_mats(u["acts"][1]["kw"], u["acts"][1]["kb"])
        out.append(
            dict(
                Wd=W0[:ci16], Ws=W0[ci16:], W1=W1, W2U0a=W2 @ U0[ci16:],
                U0x=U0[:ci16], U1=U1, U2=U2,
                GRm1=GRm1, bm1=bm1,
                kw2=np.asarray(m["acts"][1]["kw"], np.float32),
                kb2=np.asarray(m["acts"][1]["kb"], np.float32),
                GRu1=GRu1, bu1=bu1, GRu2=GRu2, bu2=bu2,
            )
        )
    return out


def _split_bf16(x):
    import ml_dtypes

    x = np.asarray(x, np.float32)
    hi = x.astype(ml_dtypes.bfloat16)
    lo = (x - hi.astype(np.float32)).astype(ml_dtypes.bfloat16)
    return np.concatenate([hi, lo], axis=-1)


def _wrap_idx(idx, G):
    a = np.asarray(idx, np.int16).reshape(G // 16, 16).T  # [16, G/16]
    return np.ascontiguousarray(np.tile(a, (8, 1)))


def _preprocess_edges(edge_index):
    src = np.asarray(edge_index[0], np.int64)
    dst = np.asarray(edge_index[1], np.int64)
    order = np.argsort(dst, kind="stable")
    srcS, dstS = src[order], dst[order]
    NTILES = (N_NODES + 127) // 128

    tile_of = dstS // 128
    tile_counts = np.bincount(tile_of, minlength=NTILES)
    tile_starts = np.concatenate([[0], np.cumsum(tile_counts)])

    bounds = [0]
    cum = np.cumsum(tile_counts)
    for k in range(1, NCORES):
        bounds.append(int(np.searchsorted(cum, k * N_EDGES / NCORES)))
    bounds.append(NTILES)
    core_tiles = [(bounds[k], bounds[k + 1]) for k in range(NCORES)]
    maxtiles = max(b - a for a, b in core_tiles)

    per_core = []
    for k, (t0, t1) in enumerate(core_tiles):
        segs = []
        for j, t in enumerate(range(t0, t1)):
            s, e = int(tile_starts[t]), int(tile_starts[t + 1])
            cnt = e - s
            pad = (-cnt) % GRP
            segs.append(
                dict(
                    tile_slot=j,
                    src=np.concatenate([srcS[s:e], np.zeros(pad, np.int64)]),
                    dst_local=np.concatenate(
                        [dstS[s:e] - t * 128, -np.ones(pad, np.int64)]
                    ),
                    dst_table=np.concatenate(
                        [dstS[s:e] - t0 * 128, np.zeros(pad, np.int64)]
                    ),
                )
            )
        per_core.append(dict(t0=t0, t1=t1, segs=segs))
    G = max(sum(len(s["src"]) for s in pc["segs"]) for pc in per_core)
    G = ((G + GCH - 1) // GCH) * GCH
    for pc in per_core:
        cur = sum(len(s["src"]) for s in pc["segs"])
        if G - cur:
            pc["segs"].append(
                dict(
                    tile_slot=0,
                    src=np.zeros(G - cur, np.int64),
                    dst_local=-np.ones(G - cur, np.int64),
                    dst_table=np.zeros(G - cur, np.int64),
                )
            )
    return per_core, maxtiles, G


_HOOK_DONE = False


def _install_ntff_hook():
    global _HOOK_DONE
    if _HOOK_DONE:
        return
    _HOOK_DONE = True
    try:
        import antenv
        from trn_agent_boot.trn_boot import _ntff_profile_via_ctypes

        mod = types.ModuleType("antenv.axon_hooks")
        _h = [None]
        mod.set_axon_ntff_profile_hook = lambda v: _h.__setitem__(0, v)
        mod.get_axon_ntff_profile_hook = lambda: _h[0]
        sys.modules["antenv.axon_hooks"] = mod
        antenv.axon_hooks = mod
        mod.set_axon_ntff_profile_hook(
            _ntff_profile_via_ctypes("/opt/axon/libaxon_pjrt.so")
        )
    except Exception:
        pass


def build_bass(maxtiles, G, use_f32r=True, use_ag=True, use_scat=True, nlayers=3):
    import concourse.bass as bass
    import concourse.tile as tile
    from concourse import bacc, mybir
    from contextlib import ExitStack

    F32 = mybir.dt.float32
    F32R = mybir.dt.float32r if use_f32r else mybir.dt.float32
    BF16 = mybir.dt.bfloat16
    I16 = mybir.dt.int16
    I32 = mybir.dt.int32
    SIG = mybir.ActivationFunctionType.Sigmoid

    LN = maxtiles * 128
    LNP = ((LN + GRP - 1) // GRP) * GRP
    NG = LNP // GRP
    GG = G // GRP
    NCH = G // GCH
    B0ROWS = ((N_NODES + 127) // 128) * 128
    BAGROWS = NCORES * LN

    nc = bacc.Bacc("TRN2", num_devices=NCORES, debug=False)

    def din(name, shape, dt):
        return nc.dram_tensor(name, shape, dt, kind="ExternalInput")

    xloc = din("xloc", [LN, 128], F32)
    b0 = din("b0", [B0ROWS, 512], BF16)
    a0 = din("a0", [LN, 512], BF16)
    idxA = din("idxA", [128, G // 16], I16)
    idxB0 = din("idxB0", [128, G // 16], I16)
    idxB12 = din("idxB12", [128, G // 16], I16)
    dstw_in = din("dstw", [128, G // 128], F32)
    idxF = din("idxF", [128, GG * 8], I16)
    iota_in = din("iota", [128, 128], F32)
    ident_in = din("ident", [128, 128], F32)
    WROWS = [2 * D_LAYERS[l] + 4 * H + D_LAYERS[l] for l in range(3)]
    wpk = [din(f"wpk{l}", [WROWS[l], 256], F32) for l in range(3)]
    grpk = [din(f"grpk{l}", [6 * 128, 128], F32) for l in range(3)]
    mpk = [din(f"mpk{l}", [128, 6 + 256 + 16], F32) for l in range(3)]

    out_rows = nc.dram_tensor("out_rows", [LN, 128], F32, kind="ExternalOutput")

    with tile.TileContext(nc) as tc, ExitStack() as ctx:
        cpool = ctx.enter_context(tc.tile_pool(name="const", bufs=1))
        wpool = ctx.enter_context(tc.tile_pool(name="work", bufs=2))
        gpool = ctx.enter_context(tc.tile_pool(name="gath", bufs=3))
        ppool = ctx.enter_context(tc.tile_pool(name="ps1", bufs=1, space="PSUM"))
        ppool2 = ctx.enter_context(tc.tile_pool(name="ps2", bufs=2, space="PSUM"))

        dpool = ctx.enter_context(tc.tile_pool(name="dram", bufs=1, space="DRAM"))
        a_scr = dpool.tile([LN, 512], BF16, name="a_scr", tag="a_scr")
        bloc = dpool.tile([LN, 512], BF16, name="bloc", tag="bloc")
        bag = [
            dpool.tile([BAGROWS, 512], BF16, addr_space="Shared",
                       name=f"bag{i}", tag=f"bag{i}")
            for i in range(2)
        ]
        agg_d = dpool.tile([LN, 256], F32, name="agg_d", tag="agg_d")

        iota_t = cpool.tile([128, 128], F32)
        ident_t = cpool.tile([128, 128], F32)
        nc.sync.dma_start(iota_t[:], iota_in[:, :])
        nc.sync.dma_start(ident_t[:], ident_in[:, :])
        idxA_t = cpool.tile([128, G // 16], I16)
        idxB0_t = cpool.tile([128, G // 16], I16)
        idxB12_t = cpool.tile([128, G // 16], I16)
        nc.sync.dma_start(idxA_t[:], idxA[:, :])
        nc.sync.dma_start(idxB0_t[:], idxB0[:, :])
        nc.sync.dma_start(idxB12_t[:], idxB12[:, :])
        dstw_t = cpool.tile([128, G // 128], F32)
        nc.sync.dma_start(dstw_t[:], dstw_in[:, :])
        idxF_t = cpool.tile([128, GG * 8], I16)
        nc.sync.dma_start(idxF_t[:], idxF[:, :])

        RMAX = max(WROWS) // 128

        def load_weights(l):
            R = WROWS[l] // 128
            raw = cpool.tile([128, RMAX * 256], F32, tag="stage", name=f"wraw{l}")
            nc.sync.dma_start(
                raw[:, : R * 256].rearrange("p (r c) -> p r c", c=256),
                wpk[l][:, :].rearrange("(r p) c -> p r c", p=128),
            )
            wr = cpool.tile([128, RMAX * 256], F32R, tag="wr", name=f"wr{l}")
            nc.vector.tensor_copy(wr[:, : R * 256], raw[:, : R * 256])
            wview = wr[:].rearrange("p (r c) -> p r c", c=256)
            graw = cpool.tile([128, 6 * 128], F32, tag="graw", name=f"graw{l}")
            nc.sync.dma_start(
                graw[:].rearrange("p (r c) -> p r c", c=128),
                grpk[l][:, :].rearrange("(r p) c -> p r c", p=128),
            )
            grr = cpool.tile([128, 6 * 128], F32R, tag="grr", name=f"grr{l}")
            nc.vector.tensor_copy(grr[:], graw[:])
            grview = grr[:].rearrange("p (r c) -> p r c", c=128)
            misc = cpool.tile([128, 6 + 256 + 16], F32, tag="misc", name=f"misc{l}")
            nc.sync.dma_start(misc[:], mpk[l][:, :])
            Dl = D_LAYERS[l]

            def seg(r0, nrows):
                return [wview[:, r0 // 128 + i, :] for i in range(nrows // 128)]

            r0 = 0
            Wd = seg(r0, Dl); r0 += Dl
            Ws = seg(r0, Dl); r0 += Dl
            W1 = seg(r0, H); r0 += H
            W2U0a = seg(r0, H); r0 += H
            U0x = seg(r0, Dl); r0 += Dl
            U1 = seg(r0, H); r0 += H
            U2 = seg(r0, H); r0 += H
            return dict(
                Wd=Wd, Ws=Ws, W1=W1, W2U0a=W2U0a, U0x=U0x, U1=U1, U2=U2,
                GRm1=[grview[:, 0, :], grview[:, 1, :]],
                GRu1=[grview[:, 2, :], grview[:, 3, :]],
                GRu2=[grview[:, 4, :], grview[:, 5, :]],
                bm1=[misc[:, 0:1], misc[:, 1:2]],
                bu1=[misc[:, 2:3], misc[:, 3:4]],
                bu2=[misc[:, 4:5], misc[:, 5:6]],
                kw2=misc[:, 6:262],
                kb2=misc[:, 262:278],
            )

        xt_a = [cpool.tile([128, LNP], F32R, tag=f"xta{i}", name=f"xta{i}") for i in range(2)]
        xt_b = [cpool.tile([128, LNP], F32R, tag=f"xtb{i}", name=f"xtb{i}") for i in range(2)]
        aggAll = cpool.tile([128, maxtiles * 256], F32, tag="aggAll")

        for i in range(2):
            nc.vector.memset(xt_a[i][:].bitcast(F32), 0.0)
            nc.vector.memset(xt_b[i][:].bitcast(F32), 0.0)
        xrows = cpool.tile([128, maxtiles * 128], F32, tag="stage", name="xrows")
        nc.sync.dma_start(
            xrows[:].rearrange("p (j c) -> p j c", c=128),
            xloc[:, :].rearrange("(j p) c -> p j c", p=128),
        )
        for j in range(maxtiles):
            pt = ppool.tile([128, 128], F32, tag="tr")
            nc.tensor.transpose(pt[:], xrows[:, j * 128 : (j + 1) * 128], ident_t[:])
            nc.vector.tensor_copy(xt_a[0][:, j * 128 : (j + 1) * 128], pt[:])

        xt_cur = xt_a

        for l in range(nlayers):
            W = load_weights(l)
            kd = D_LAYERS[l] // 128
            b_src_ap = {0: b0[:, :], 1: bag[0][:], 2: bag[1][:]}[l]
            a_src_ap = a0[:, :] if l == 0 else a_scr[:]
            idxB_t = idxB0_t if l == 0 else idxB12_t

            if l > 0:
                # build this layer's A/B tables from xt_cur, then AllGather B
                for ng in range(NG):
                    sl = slice(ng * GRP, (ng + 1) * GRP)
                    abT = {}
                    for mt in range(2):
                        for wkey in ("Wd", "Ws"):
                            pa = ppool.tile([128, GRP], F32, tag="pu")
                            for kt in range(kd):
                                nc.tensor.matmul(
                                    pa[:], W[wkey][kt][:, mt * 128 : (mt + 1) * 128],
                                    xt_cur[kt][:, sl], start=(kt == 0), stop=(kt == kd - 1),
                                )
                            t = wpool.tile(
                                [128, GRP], F32, tag=f"abT{wkey}{mt}",
                                name=f"abT{wkey}{mt}",
                            )
                            nc.vector.tensor_copy(t[:], pa[:])
                            abT[(wkey, mt)] = t
                    for jj in range(GRP // 128):
                        j = ng * (GRP // 128) + jj
                        if j >= maxtiles:
                            continue
                        for wkey, scratch in (("Wd", a_scr), ("Ws", bloc)):
                            pack = wpool.tile([128, 512], BF16, tag="pack")
                            for mt in range(2):
                                pt = ppool.tile([128, 128], F32, tag="tr")
                                nc.tensor.transpose(
                                    pt[:],
                                    abT[(wkey, mt)][:, jj * 128 : (jj + 1) * 128],
                                    ident_t[:],
                                )
                                hi = wpool.tile([128, 128], BF16, tag="hi")
                                nc.vector.tensor_copy(hi[:], pt[:])
                                lo32 = wpool.tile([128, 128], F32, tag="lo32")
                                nc.vector.tensor_sub(lo32[:], pt[:], hi[:])
                                nc.vector.tensor_copy(pack[:, mt * 128 : (mt + 1) * 128], hi[:])
                                lo = wpool.tile([128, 128], BF16, tag="lo")
                                nc.vector.tensor_copy(lo[:], lo32[:])
                                nc.vector.tensor_copy(
                                    pack[:, 256 + mt * 128 : 256 + (mt + 1) * 128], lo[:]
                                )
                            nc.sync.dma_start(
                                scratch[:].rearrange("(j p) c -> p j c", p=128)[:, j, :],
                                pack[:],
                            )
                if use_ag:
                    nc.gpsimd.collective_compute(
                        "AllGather",
                        mybir.AluOpType.bypass,
                        replica_groups=[list(range(NCORES))],
                        ins=[bloc.opt()],
                        outs=[bag[l - 1].opt()],
                    )
                else:
                    nc.sync.dma_start(
                        bag[l - 1][:].rearrange("(j p) c -> p j c", p=128)[
                            :, : LN // 128, :
                        ],
                        bloc[:].rearrange("(j p) c -> p j c", p=128),
                    )

            nc.vector.memset(aggAll[:], 0.0)
            nc.sync.dma_start(
                agg_d[:].rearrange("(j p) c -> p j c", p=128),
                aggAll[:].rearrange("p (j c) -> p j c", c=256),
            )

            for ch in range(NCH):
                gA = gpool.tile([128, 4, GCH], BF16, tag="g", name="gA")
                gB = gpool.tile([128, 4, GCH], BF16, tag="g", name="gB")
                csl = slice(ch * (GCH // 16), (ch + 1) * (GCH // 16))
                nc.gpsimd.dma_gather(
                    out_ap=gA[:], in_ap=a_src_ap, idxs_ap=idxA_t[:, csl],
                    num_idxs=GCH, num_idxs_reg=GCH, elem_size=512, transpose=True,
                )
                nc.gpsimd.dma_gather(
                    out_ap=gB[:], in_ap=b_src_ap, idxs_ap=idxB_t[:, csl],
                    num_idxs=GCH, num_idxs_reg=GCH, elem_size=512, transpose=True,
                )
                for half in range(GCH // GRP):
                    g = ch * (GCH // GRP) + half
                    sl = slice(half * GRP, (half + 1) * GRP)
                    h1 = [wpool.tile([128, GRP], F32R, tag=f"h1_{ft}", name=f"h1_{ft}") for ft in range(2)]
                    for ft in range(2):
                        ta = wpool.tile([128, GRP], F32, tag="ta")
                        tb = wpool.tile([128, GRP], F32, tag="tb")
                        nc.vector.tensor_add(ta[:], gA[:, ft, sl], gA[:, 2 + ft, sl])
                        nc.vector.tensor_add(tb[:], gB[:, ft, sl], gB[:, 2 + ft, sl])
                        nc.vector.tensor_add(h1[ft][:], ta[:], tb[:])
                    h1g = [wpool.tile([128, GRP], F32R, tag=f"h1g{ft}", name=f"h1g{ft}") for ft in range(2)]
                    for ft in range(2):
                        pl = ppool.tile([128, GRP], F32, tag="plog")
                        nc.tensor.matmul(pl[:], W["GRm1"][ft], h1[ft][:], start=True, stop=True)
                        sg = wpool.tile([128, GRP], F32, tag="sg")
                        nc.scalar.activation(sg[:], pl[:], SIG, bias=W["bm1"][ft])
                        nc.vector.tensor_mul(h1g[ft][:], h1[ft][:], sg[:])
                    h2g = []
                    for es in range(4):
                        ph2 = ppool2.tile([128, 256], F32, tag="ph2")
                        for kt in range(2):
                            nc.tensor.matmul(
                                ph2[:], h1g[kt][:, es * 128 : (es + 1) * 128],
                                W["W1"][kt], start=(kt == 0), stop=(kt == 1),
                            )
                        tm = wpool.tile([128, 256], F32, tag="tm")
                        nc.vector.tensor_mul(tm[:], ph2[:], W["kw2"])
                        gl = wpool.tile([128, 16], F32, tag="gl")
                        nc.vector.tensor_reduce(
                            gl[:], tm[:].rearrange("p (c m) -> p c m", m=16),
                            mybir.AxisListType.X, mybir.AluOpType.add,
                        )
                        gl2 = wpool.tile([128, 16], F32, tag="gl2")
                        nc.vector.tensor_add(gl2[:], gl[:], W["kb2"])
                        gs = wpool.tile([128, 16], F32, tag="gs")
                        nc.scalar.activation(gs[:], gl2[:], SIG)
                        hg = wpool.tile([128, 256], F32R, tag=f"hg{es}")
                        nc.vector.tensor_mul(
                            hg[:].rearrange("p (c m) -> p c m", m=16),
                            ph2[:].rearrange("p (c m) -> p c m", m=16),
                            gs[:].rearrange("p (c o) -> p c o", o=1).broadcast_to([128, 16, 16]),
                        )
                        h2g.append(hg)
                    pagg = ppool.tile([128, 256], F32, tag="pagg")
                    for es in range(4):
                        blk = g * 4 + es
                        S = wpool.tile([128, 128], F32R, tag="S")
                        nc.vector.tensor_tensor(
                            S[:],
                            dstw_t[:, blk : blk + 1].broadcast_to([128, 128]),
                            iota_t[:],
                            mybir.AluOpType.is_equal,
                        )
                        nc.tensor.matmul(
                            pagg[:], S[:], h2g[es][:], start=(es == 0), stop=(es == 3)
                        )
                    pcp = wpool.tile([128, 256], F32, tag="tm", name="pcp")
                    nc.vector.tensor_copy(pcp[:], pagg[:])
                    if use_scat:
                        nc.gpsimd.dma_scatter_add(
                            out_ap=agg_d[:],
                            in_ap=pcp[:].rearrange("p (b c) -> p b c", c=256),
                            idxs_ap=idxF_t[:, g * 8 : (g + 1) * 8],
                            num_idxs=128,
                            num_idxs_reg=128,
                            elem_size=256,
                        )
                    else:
                        nc.sync.dma_start(
                            agg_d[:].rearrange("(j p) c -> p j c", p=128)[:, 0, :],
                            pcp[:],
                        )

            nc.sync.dma_start(
                aggAll[:].rearrange("p (j c) -> p j c", c=256),
                agg_d[:].rearrange("(j p) c -> p j c", p=128),
            )
            aggT = [cpool.tile([128, LNP], F32R, tag=f"aggT{kt}", name=f"aggT{kt}") for kt in range(2)]
            for kt in range(2):
                nc.vector.memset(aggT[kt][:].bitcast(F32), 0.0)
            for j in range(maxtiles):
                for kt in range(2):
                    pt = ppool.tile([128, 128], F32, tag="tr")
                    nc.tensor.transpose(
                        pt[:],
                        aggAll[:, j * 256 + kt * 128 : j * 256 + (kt + 1) * 128],
                        ident_t[:],
                    )
                    nc.vector.tensor_copy(aggT[kt][:, j * 128 : (j + 1) * 128], pt[:])

            xt_nxt = xt_b if l % 2 == 0 else xt_a
            Dn = D_LAYERS[l + 1]
            for ng in range(NG):
                sl = slice(ng * GRP, (ng + 1) * GRP)
                u1g = [wpool.tile([128, GRP], F32R, tag=f"u1g{mt}", name=f"u1g{mt}") for mt in range(2)]
                for mt in range(2):
                    pu = ppool.tile([128, GRP], F32, tag="pu")
                    for kt in range(kd):
                        nc.tensor.matmul(
                            pu[:], W["U0x"][kt][:, mt * 128 : (mt + 1) * 128],
                            xt_cur[kt][:, sl], start=(kt == 0), stop=False,
                        )
                    for kt in range(2):
                        nc.tensor.matmul(
                            pu[:], W["W2U0a"][kt][:, mt * 128 : (mt + 1) * 128],
                            aggT[kt][:, sl], start=False, stop=(kt == 1),
                        )
                    u1 = wpool.tile([128, GRP], F32R, tag="ust")
                    nc.vector.tensor_copy(u1[:], pu[:])
                    plg = ppool.tile([128, GRP], F32, tag="plog")
                    nc.tensor.matmul(plg[:], W["GRu1"][mt], u1[:], start=True, stop=True)
                    sg = wpool.tile([128, GRP], F32, tag="usg")
                    nc.scalar.activation(sg[:], plg[:], SIG, bias=W["bu1"][mt])
                    nc.vector.tensor_mul(u1g[mt][:], u1[:], sg[:])
                u2g = [wpool.tile([128, GRP], F32R, tag=f"u2g{mt}", name=f"u2g{mt}") for mt in range(2)]
                for mt in range(2):
                    pu = ppool.tile([128, GRP], F32, tag="pu")
                    for kt in range(2):
                        nc.tensor.matmul(
                            pu[:], W["U1"][kt][:, mt * 128 : (mt + 1) * 128],
                            u1g[kt][:, :], start=(kt == 0), stop=(kt == 1),
                        )
                    u2 = wpool.tile([128, GRP], F32R, tag="ust")
                    nc.vector.tensor_copy(u2[:], pu[:])
                    plg = ppool.tile([128, GRP], F32, tag="plog")
                    nc.tensor.matmul(plg[:], W["GRu2"][mt], u2[:], start=True, stop=True)
                    sg = wpool.tile([128, GRP], F32, tag="usg")
                    nc.scalar.activation(sg[:], plg[:], SIG, bias=W["bu2"][mt])
                    nc.vector.tensor_mul(u2g[mt][:], u2[:], sg[:])
                for mt in range(Dn // 128):
                    pu = ppool.tile([128, GRP], F32, tag="pu")
                    for kt in range(2):
                        nc.tensor.matmul(
                            pu[:], W["U2"][kt][:, mt * 128 : (mt + 1) * 128],
                            u2g[kt][:, :], start=(kt == 0), stop=(kt == 1),
                        )
                    nc.vector.tensor_copy(xt_nxt[mt][:, sl], pu[:])

            if l == nlayers - 1:
                orows = cpool.tile([128, maxtiles * 128], F32, tag="orows")
                xv = xt_nxt[0][:].bitcast(F32)
                for j in range(maxtiles):
                    pt = ppool.tile([128, 128], F32, tag="tr")
                    nc.tensor.transpose(pt[:], xv[:, j * 128 : (j + 1) * 128], ident_t[:])
                    nc.vector.tensor_copy(orows[:, j * 128 : (j + 1) * 128], pt[:])
                nc.sync.dma_start(
                    out_rows[:, :].rearrange("(j p) c -> p j c", p=128),
                    orows[:].rearrange("p (j c) -> p j c", c=128),
                )

            xt_cur = xt_nxt

    nc.compile()
    return nc


def make_in_maps(x, edge_index, params, maxtiles=None, G=None, per_core=None):
    import ml_dtypes

    x = np.asarray(x, np.float32)
    P = _prep_params(params)
    if per_core is None:
        per_core, maxtiles, G = _preprocess_edges(edge_index)
    LN = maxtiles * 128
    GG = G // GRP
    B0ROWS = ((N_NODES + 127) // 128) * 128

    X0 = x.reshape(N_NODES, 128)
    X0p = np.zeros((B0ROWS, 128), np.float32)
    X0p[:N_NODES] = X0
    B0 = _split_bf16(X0p @ P[0]["Ws"]).astype(ml_dtypes.bfloat16)

    node_core = np.zeros(N_NODES, np.int64)
    for k, pc in enumerate(per_core):
        node_core[pc["t0"] * 128 : min(pc["t1"] * 128, N_NODES)] = k
    core_base = np.array([pc["t0"] * 128 for pc in per_core])

    shared = {}
    for l in range(3):
        shared[f"wpk{l}"] = np.concatenate(
            [
                P[l]["Wd"], P[l]["Ws"], P[l]["W1"], P[l]["W2U0a"],
                P[l]["U0x"], P[l]["U1"],
                np.pad(P[l]["U2"], [(0, 0), (0, 256 - P[l]["U2"].shape[1])]),
            ]
        ).astype(np.float32)
        grs = []
        for GR in (P[l]["GRm1"], P[l]["GRu1"], P[l]["GRu2"]):
            for mt in range(2):
                grs.append(GR[mt * 128 : (mt + 1) * 128, mt * 128 : (mt + 1) * 128])
        shared[f"grpk{l}"] = np.concatenate(grs).astype(np.float32)
        misc = np.zeros((128, 6 + 256 + 16), np.float32)
        for bi, b in enumerate((P[l]["bm1"], P[l]["bu1"], P[l]["bu2"])):
            misc[:, 2 * bi] = b[:128]
            misc[:, 2 * bi + 1] = b[128:]
        misc[:, 6:262] = np.tile(P[l]["kw2"].reshape(1, 256), (128, 1))
        misc[:, 262:278] = np.tile(P[l]["kb2"][None, :], (128, 1))
        shared[f"mpk{l}"] = misc
    shared["iota"] = np.tile(np.arange(128, dtype=np.float32)[None, :], (128, 1))
    shared["ident"] = np.eye(128, dtype=np.float32)
    shared["b0"] = B0

    in_maps = []
    for k, pc in enumerate(per_core):
        t0, t1 = pc["t0"], pc["t1"]
        nreal = min(t1 * 128, N_NODES) - t0 * 128
        xl = np.zeros((LN, 128), np.float32)
        xl[:nreal] = X0[t0 * 128 : t0 * 128 + nreal]
        A0 = _split_bf16(xl @ P[0]["Wd"]).astype(ml_dtypes.bfloat16)

        src = np.concatenate([s["src"] for s in pc["segs"]])
        dstl = np.concatenate([s["dst_local"] for s in pc["segs"]])
        dstt = np.concatenate([s["dst_table"] for s in pc["segs"]])
        slots = np.concatenate(
            [np.full(len(s["src"]), s["tile_slot"], np.int64) for s in pc["segs"]]
        )
        assert len(src) == G

        src_ag = node_core[src] * LN + (src - core_base[node_core[src]])
        gslots = slots[::GRP]
        assert len(gslots) == GG
        idxf = (
            gslots[:, None] * 128 + np.arange(128)[None, :]
        ).reshape(-1)  # (GG*128,)
        idxF = _wrap_idx(idxf, GG * 128)

        im = dict(
            xloc=xl,
            a0=A0,
            idxA=_wrap_idx(dstt, G),
            idxB0=_wrap_idx(src, G),
            idxB12=_wrap_idx(src_ag, G),
            dstw=np.ascontiguousarray(
                dstl.reshape(G // 128, 128).T.astype(np.float32)
            ),
            idxF=idxF,
            **shared,
        )
        in_maps.append(im)
    return in_maps, per_core, maxtiles, G


def kernel(x, edge_index, params):
    _install_ntff_hook()
    from concourse import bass_utils

    in_maps, per_core, maxtiles, G = make_in_maps(x, edge_index, params)
    nc = build_bass(maxtiles, G, use_f32r=True)
    res = bass_utils.run_bass_kernel_spmd(
        nc, in_maps, core_ids=list(range(NCORES)), trace=False
    )
    out = np.zeros((N_NODES, 128), np.float32)
    for k, pc in enumerate(per_core):
        t0 = pc["t0"]
        nreal = min(pc["t1"] * 128, N_NODES) - t0 * 128
        out[t0 * 128 : t0 * 128 + nreal] = res.results[k]["out_rows"][:nreal]
    return out.reshape(N_NODES, 8, 16)
